# revision 2
# baseline (speedup 1.0000x reference)
"""Band-decomposition GAT kernel for 8 trn2 NeuronCores.

Math (reference):
    Wx = x @ W;  s1 = Wx@a1/s;  s2 = Wx@a2/s   (s = sqrt(2D), per t)
    weight = softmax_m(lrelu(s1[m] + s2[n]));  agg = lrelu(weight @ Wx)
    out = x - agg

Key identities (per t):
  * Rescaling the softmax row by exp(-s2[n]):
        E~[m,n] = max(E1[m], F1[m] * r[n]),
    E1 = exp(s1), F1 = exp(0.01 s1), r = exp(-0.99 s2); the branch flips
    exactly at s1[m] >= -s2[n] =: tau[n].
  * With m SORTED by s1 and dest n sorted by tau, k(n) = searchsorted
    (s1_sorted, tau[n]) is monotone: all m below k are in the F1*r branch,
    all above in the E1 branch. For a 128-dest tile whose k-range fits in a
    static window of blocks [B_j, B_j+C_j):
        num[n,:] = r[n]*Hpre[128 B_j] + Gsuf[128(B_j+C_j)]
                   + sum_{in band} max(E1, F1 r[n]) Wx[m,:]
    where Gpre/Hpre/Gsuf are prefix/suffix sums of E1*Wx / F1*Wx over
    sorted m -- and since prefix commutes with @W, the HOST computes the
    coarse rows exactly (they are (cumsum of E1*x) @ W).
  * den[n] is a pure function of (s1, s2): host computes u = 1/den.
  * Odd cores take the HIGH-tau dest half with the m-axis sort MIRRORED,
    so the same static band schedule serves all 8 SPMD cores.

Device per core (t, half): project the sorted band blocks (Wx), build the
dense band scores with one fused DVE op per block span, accumulate
num^T[d, n] via per-block matmuls + one rank-2 matmul per tile for the
coarse terms, then out^T = xdest^T - lrelu(num^T * u).  Host transposes
and unpermutes the [128, 2048] per-core result.
"""

import sys

if "/opt/trn_rl_repo" not in sys.path:
    sys.path.insert(0, "/opt/trn_rl_repo")

import numpy as np
import ml_dtypes

_bf16 = ml_dtypes.bfloat16

N, T, D = 4096, 4, 128
P = 128
HALF = N // 2
NTILE = HALF // P          # 16 dest tiles per core
SCALE = (2.0 * D) ** 0.5

_CACHE = {}


def _schedule(kmins, kmaxs):
    """Static per-tile band windows from cross-core k ranges.

    kmins/kmaxs: [8, NTILE] arrays. Returns tuple of (B_j, C_j)."""
    lo = kmins.min(axis=0)
    hi = kmaxs.max(axis=0)
    sched = []
    for j in range(NTILE):
        b = int(lo[j]) // P
        c = -(-int(hi[j]) // P) - b  # ceil
        c = max(c, 1)
        sched.append((b, c))
    return tuple(sched)


def _build(sched):
    import concourse.mybir as mybir
    from concourse import bacc
    from concourse.tile import TileContext

    f32 = mybir.dt.float32
    b16 = mybir.dt.bfloat16
    Alu = mybir.AluOpType
    Act = mybir.ActivationFunctionType

    nblk = max(b + c for b, c in sched)
    # block B -> contiguous run of tiles [jlo, jhi] that use it
    users = {}
    for j, (b, c) in enumerate(sched):
        for bb in range(b, b + c):
            lo, hi = users.get(bb, (j, j))
            users[bb] = (min(lo, j), max(hi, j))

    nc = bacc.Bacc()
    xtb = nc.declare_dram_parameter("xtb", [P, nblk * P], b16, isOutput=False)
    wb = nc.declare_dram_parameter("wb", [P, P], b16, isOutput=False)
    ef = nc.declare_dram_parameter("ef", [P, 2 * nblk], f32, isOutput=False)
    rb = nc.declare_dram_parameter("rb", [P, HALF], b16, isOutput=False)
    ub = nc.declare_dram_parameter("ub", [P, HALF], b16, isOutput=False)
    cst = nc.declare_dram_parameter("cst", [8, 4 * P], b16, isOutput=False)
    cmv = nc.declare_dram_parameter("cmv", [8, HALF], b16, isOutput=False)
    xtd = nc.declare_dram_parameter("xtd", [P, HALF], f32, isOutput=False)
    out = nc.declare_dram_parameter("out", [P, HALF], f32, isOutput=True)

    ngrp = -(-nblk // 4)

    with TileContext(nc) as tc:
        with (
            tc.tile_pool(name="const", bufs=1) as cp,
            tc.tile_pool(name="et", bufs=4) as ep,
            tc.tile_pool(name="fin", bufs=2) as fp,
        ):
            # ---- input DMAs, spread across issue engines so DGE setups
            # overlap and the projection starts early ----
            wb_sb = cp.tile([P, P], b16)
            nc.scalar.dma_start(wb_sb[:, :], wb[:, :])
            # xtb in two large pieces, one per HWDGE ring, so the transfers
            # overlap and projection is fed without per-chunk stalls
            xtb_chunks = []
            for g in range(ngrp):
                w = min(512, nblk * P - g * 512)
                ch = cp.tile([P, w], b16, name=f"xtb{g}", tag=f"xtb{g}")
                nc.sync.dma_start(ch[:, :], xtb[:, g * 512 : g * 512 + w])
                xtb_chunks.append(ch)

            def xtb_blk(blk):
                return xtb_chunks[blk // 4][:, (blk % 4) * P : (blk % 4) * P + P]

            rb_sb = cp.tile([P, HALF], b16)
            nc.scalar.dma_start(rb_sb[:, :], rb[:, :])
            ef_sb = cp.tile([P, 2 * nblk], f32)
            nc.gpsimd.dma_start(ef_sb[:, :], ef[:, :])
            cst_sb = cp.tile([8, 4 * P], b16)
            nc.gpsimd.dma_start(cst_sb[:, :], cst[:, :])
            cmv_sb = cp.tile([8, HALF], b16)
            nc.gpsimd.dma_start(cmv_sb[:, :], cmv[:, :])
            ub_sb = cp.tile([P, HALF], b16)
            nc.gpsimd.dma_start(ub_sb[:, :], ub[:, :])
            xtd_sb = cp.tile([P, HALF], f32)
            nc.gpsimd.dma_start(xtd_sb[:, :], xtd[:, :])



            wxs = cp.tile([P, nblk * P], b16)

            with (
                tc.tile_pool(name="pproj", bufs=2, space="PSUM") as pp,
                tc.tile_pool(name="pnum", bufs=1, space="PSUM") as pn,
            ):
                # ---- num^T accumulator: 4 psum banks ----
                num = [pn.tile([P, 512], f32, name=f"num{q}") for q in range(4)]

                # ---- projection: wx rows per block (4 blocks / bank);
                # evacuate on ACT so the DVE is free for et tiles ----
                for g in range(ngrp):
                    ps = pp.tile([P, 512], f32, name="projps", tag="pj")
                    for c in range(min(4, nblk - 4 * g)):
                        nc.tensor.matmul(
                            ps[:, c * P : (c + 1) * P],
                            xtb_blk(4 * g + c),
                            wb_sb[:, :],
                            start=True,
                            stop=True,
                        )
                    w = min(512, (nblk - 4 * g) * P)
                    nc.scalar.activation(
                        wxs[:, g * 512 : g * 512 + w], ps[:, :w], Act.Copy
                    )

                # ---- bank-major: per psum bank, coarse rank-8 matmul
                # (start=True initializes the bank), band segments, then the
                # finalize chain -- so bank q's output streams out while
                # bank q+1 still computes ----
                et_tiles = {}
                for blk in sorted(users):
                    jlo, jhi = users[blk]
                    et_tiles[blk] = (jlo * P, (jhi + 1) * P)

                for q in range(4):
                    nc.tensor.matmul(
                        num[q][:, :],
                        cst_sb[:, q * P : (q + 1) * P],
                        cmv_sb[:, q * 512 : (q + 1) * 512],
                        start=True,
                        stop=False,
                        skip_group_check=True,
                    )
                    # blocks whose tile span intersects this bank
                    q_lo, q_hi = q * 512, (q + 1) * 512
                    blks = [
                        b for b, (lo_n, hi_n) in et_tiles.items()
                        if lo_n < q_hi and hi_n > q_lo
                    ]
                    for bi, blk in enumerate(blks):
                        lo_n, hi_n = et_tiles[blk]
                        c0, c1 = max(lo_n, q_lo), min(hi_n, q_hi)
                        et = ep.tile([P, 512], b16, name="et")
                        nc.vector.tensor_scalar(
                            et[:, : c1 - c0],
                            rb_sb[:, c0:c1],
                            ef_sb[:, nblk + blk : nblk + blk + 1],
                            ef_sb[:, blk : blk + 1],
                            Alu.mult,
                            Alu.max,
                        )
                        nc.tensor.matmul(
                            num[q][:, c0 - q_lo : c1 - q_lo],
                            wxs[:, blk * P : (blk + 1) * P],
                            et[:, : c1 - c0],
                            start=False,
                            stop=(bi == len(blks) - 1),
                            skip_group_check=True,
                        )

                    # finalize this bank: out^T = xtd - lrelu(num)*u
                    # (u > 0 so lrelu and the u-scale commute; lrelu on DVE
                    # as (num*0.01) max num -- no ACT table switch)
                    sl = slice(q * 512, (q + 1) * 512)
                    agg = fp.tile([P, 512], f32, name="agg", tag="agg")
                    nc.vector.tensor_tensor(
                        agg[:, :], num[q][:, :], ub_sb[:, sl], Alu.mult
                    )
                    m1 = fp.tile([P, 512], f32, name="m1", tag="m1")
                    nc.scalar.activation(m1[:, :], agg[:, :], Act.Lrelu, alpha=0.01)
                    o_t = fp.tile([P, 512], f32, name="o", tag="o")
                    sub_eng = nc.gpsimd if q % 2 == 0 else nc.vector
                    sub_eng.tensor_tensor(
                        o_t[:, :], xtd_sb[:, sl], m1[:, :], Alu.subtract
                    )
                    eng = nc.sync if q % 2 == 0 else nc.scalar
                    eng.dma_start(out[:, sl], o_t[:, :])

    nc.compile()
    return nc


def _prep(x, W, a1, a2):
    """Host prep: sorting, prefix tables, packing. Returns (in_maps,
    sched, scatter) where scatter[c] = dest node ids per core."""
    x = np.asarray(x, dtype=np.float32)
    W = np.asarray(W, dtype=np.float32)
    a1 = np.asarray(a1, dtype=np.float32)
    a2 = np.asarray(a2, dtype=np.float32)
    w1 = W @ a1 / SCALE
    w2 = W @ a2 / SCALE
    Wb16 = W.astype(_bf16)

    cores = []
    kmins = np.zeros((8, NTILE), dtype=np.int64)
    kmaxs = np.zeros((8, NTILE), dtype=np.int64)
    for t in range(T):
        xt = x[:, t, :]
        s1 = xt @ w1
        s2 = xt @ w2
        tau = -s2
        for h in range(2):
            c = 2 * t + h
            if h == 0:
                order = np.argsort(s1, kind="stable")
            else:
                order = np.argsort(-s1, kind="stable")
            s1s = s1[order]
            if h == 0:
                kfull = np.searchsorted(s1s, tau, side="left")
            else:
                kfull = np.searchsorted(-s1s, -tau, side="left")
            trank = np.argsort(tau, kind="stable")
            dest = trank[:HALF] if h == 0 else trank[HALF:]
            k = kfull[dest]
            dsort = dest[np.argsort(k, kind="stable")]
            k = kfull[dsort]
            for j in range(NTILE):
                kk = k[j * P : (j + 1) * P]
                kmins[c, j] = kk.min()
                kmaxs[c, j] = kk.max()
            cores.append((c, h, order, s1s, dsort, k, xt, s2))

    sched = _schedule(kmins, kmaxs)
    nblk = max(b + cc for b, cc in sched)

    in_maps = [None] * 8
    scatter = [None] * 8
    for (c, h, order, s1s, dsort, k, xt, s2) in cores:
        E1 = np.exp(s1s)
        F1 = np.exp(0.01 * s1s)
        xs = xt[order]                                  # [N, D] sorted
        # coarse prefix tables in f64 for exactness
        GX = np.cumsum(E1[:, None] * xs, axis=0, dtype=np.float64)
        HX = np.cumsum(F1[:, None] * xs, axis=0, dtype=np.float64)
        GX = np.vstack([np.zeros((1, D)), GX])
        HX = np.vstack([np.zeros((1, D)), HX])
        ge = np.concatenate([[0.0], np.cumsum(E1, dtype=np.float64)])
        fe = np.concatenate([[0.0], np.cumsum(F1, dtype=np.float64)])

        r = np.exp(-0.99 * s2[dsort])                   # [2048]
        # exact den on host: for h=0, m<k -> F1*r ; else E1
        if h == 0:
            den = fe[k] * r + (ge[N] - ge[k])
        else:
            den = ge[k] + (fe[N] - fe[k]) * r
        u = 1.0 / den

        # coarse rows per tile at the static band edges, packed for one
        # rank-8 matmul per psum bank: cst[2*jj:2*jj+2, bank*P:...] holds
        # tile (4*bank+jj)'s (g_row, h_row); cmv rows 2*jj:2*jj+2 are
        # (ones, r) on that tile's columns and zero elsewhere.
        cst_arr = np.zeros((8, 4 * P), dtype=np.float64)
        cmv_arr = np.zeros((8, HALF), dtype=np.float32)
        for j, (b, cc) in enumerate(sched):
            lo, hi = P * b, P * (b + cc)
            kk = k[j * P : (j + 1) * P]
            assert kk.min() >= lo and kk.max() <= hi, (c, j, lo, hi, kk.min(), kk.max())
            if h == 0:
                g_row = (GX[N] - GX[hi]) @ W            # E1 branch above band
                h_row = HX[lo] @ W                      # F1 branch below band
            else:
                g_row = GX[lo] @ W
                h_row = (HX[N] - HX[hi]) @ W
            q, jj = j // 4, j % 4
            cst_arr[2 * jj, q * P : (q + 1) * P] = g_row
            cst_arr[2 * jj + 1, q * P : (q + 1) * P] = h_row
            cmv_arr[2 * jj, j * P : (j + 1) * P] = 1.0
            cmv_arr[2 * jj + 1, j * P : (j + 1) * P] = r[j * P : (j + 1) * P]

        xtb = np.zeros((P, nblk * P), dtype=_bf16)
        xtb[:, : nblk * P] = xs[: nblk * P].T.astype(_bf16)
        ef = np.zeros((P, 2 * nblk), dtype=np.float32)
        for blk in range(nblk):
            ef[:, blk] = E1[blk * P : (blk + 1) * P]
            ef[:, nblk + blk] = F1[blk * P : (blk + 1) * P]
        rb_arr = np.ascontiguousarray(np.broadcast_to(r.astype(_bf16), (P, HALF)))
        ub_arr = np.ascontiguousarray(np.broadcast_to(u.astype(_bf16), (P, HALF)))
        xtd = np.ascontiguousarray(xt[dsort].T)

        in_maps[c] = {
            "xtb": xtb,
            "wb": np.ascontiguousarray(Wb16),
            "ef": ef,
            "rb": rb_arr,
            "ub": ub_arr,
            "cst": cst_arr.astype(_bf16),
            "cmv": cmv_arr.astype(_bf16),
            "xtd": xtd,
        }
        scatter[c] = dsort
    return in_maps, sched, scatter


def _run(x, W, a1, a2, trace=False):
    from concourse.bass_utils import run_bass_kernel_spmd

    in_maps, sched, scatter = _prep(x, W, a1, a2)
    if sched not in _CACHE:
        _CACHE[sched] = _build(sched)
    nc = _CACHE[sched]
    res = run_bass_kernel_spmd(nc, in_maps, list(range(8)), trace=trace)
    out_full = np.empty((N, T, D), dtype=np.float32)
    for c in range(8):
        t = c // 2
        out_full[scatter[c], t, :] = np.asarray(
            res.results[c]["out"], dtype=np.float32
        ).T
    return out_full, res


def kernel(x, W, a1, a2):
    out, _ = _run(x, W, a1, a2, trace=False)
    return out


# revision 3
# speedup vs baseline: 1.0262x; 1.0262x over previous
"""Band-decomposition GAT kernel for 8 trn2 NeuronCores.

Math (reference):
    Wx = x @ W;  s1 = Wx@a1/s;  s2 = Wx@a2/s   (s = sqrt(2D), per t)
    weight = softmax_m(lrelu(s1[m] + s2[n]));  agg = lrelu(weight @ Wx)
    out = x - agg

Key identities (per t):
  * Rescaling the softmax row by exp(-s2[n]):
        E~[m,n] = max(E1[m], F1[m] * r[n]),
    E1 = exp(s1), F1 = exp(0.01 s1), r = exp(-0.99 s2); the branch flips
    exactly at s1[m] >= -s2[n] =: tau[n].
  * With m SORTED by s1 and dest n sorted by tau, k(n) = searchsorted
    (s1_sorted, tau[n]) is monotone: all m below k are in the F1*r branch,
    all above in the E1 branch. For a 128-dest tile whose k-range fits in a
    static window of blocks [B_j, B_j+C_j):
        num[n,:] = r[n]*Hpre[128 B_j] + Gsuf[128(B_j+C_j)]
                   + sum_{in band} max(E1, F1 r[n]) Wx[m,:]
    where Gpre/Hpre/Gsuf are prefix/suffix sums of E1*Wx / F1*Wx over
    sorted m -- and since prefix commutes with @W, the HOST computes the
    coarse rows exactly (they are (cumsum of E1*x) @ W).
  * den[n] is a pure function of (s1, s2): host computes u = 1/den.
  * Odd cores take the HIGH-tau dest half with the m-axis sort MIRRORED,
    so the same static band schedule serves all 8 SPMD cores.

Device per core (t, half): project the sorted band blocks (Wx), build the
dense band scores with one fused DVE op per block span, accumulate
num^T[d, n] via per-block matmuls + one rank-2 matmul per tile for the
coarse terms, then out^T = xdest^T - lrelu(num^T * u).  Host transposes
and unpermutes the [128, 2048] per-core result.
"""

import sys

if "/opt/trn_rl_repo" not in sys.path:
    sys.path.insert(0, "/opt/trn_rl_repo")

import numpy as np
import ml_dtypes

_bf16 = ml_dtypes.bfloat16

N, T, D = 4096, 4, 128
P = 128
HALF = N // 2
NTILE = HALF // P          # 16 dest tiles per core
SCALE = (2.0 * D) ** 0.5

_CACHE = {}


def _schedule(kmins, kmaxs):
    """Static per-tile band windows from cross-core k ranges.

    kmins/kmaxs: [8, NTILE] arrays. Returns tuple of (B_j, C_j)."""
    lo = kmins.min(axis=0)
    hi = kmaxs.max(axis=0)
    sched = []
    for j in range(NTILE):
        b = int(lo[j]) // P
        c = -(-int(hi[j]) // P) - b  # ceil
        c = max(c, 1)
        sched.append((b, c))
    return tuple(sched)


def _build(sched):
    import concourse.mybir as mybir
    from concourse import bacc
    from concourse.tile import TileContext

    f32 = mybir.dt.float32
    b16 = mybir.dt.bfloat16
    Alu = mybir.AluOpType
    Act = mybir.ActivationFunctionType

    nblk = max(b + c for b, c in sched)
    # block B -> contiguous run of tiles [jlo, jhi] that use it
    users = {}
    for j, (b, c) in enumerate(sched):
        for bb in range(b, b + c):
            lo, hi = users.get(bb, (j, j))
            users[bb] = (min(lo, j), max(hi, j))

    nc = bacc.Bacc()
    xtb = nc.declare_dram_parameter("xtb", [P, nblk * P], b16, isOutput=False)
    wb = nc.declare_dram_parameter("wb", [P, P], b16, isOutput=False)
    ef = nc.declare_dram_parameter("ef", [P, 2 * nblk], f32, isOutput=False)
    rb = nc.declare_dram_parameter("rb", [P, HALF], b16, isOutput=False)
    ub = nc.declare_dram_parameter("ub", [P, HALF], b16, isOutput=False)
    cst = nc.declare_dram_parameter("cst", [8, 4 * P], b16, isOutput=False)
    cmv = nc.declare_dram_parameter("cmv", [8, HALF], b16, isOutput=False)
    xtd = nc.declare_dram_parameter("xtd", [P, HALF], f32, isOutput=False)
    out = nc.declare_dram_parameter("out", [P, HALF], f32, isOutput=True)

    ngrp = -(-nblk // 4)

    with TileContext(nc) as tc:
        with (
            tc.tile_pool(name="const", bufs=1) as cp,
            tc.tile_pool(name="et", bufs=8) as ep,
            tc.tile_pool(name="fin", bufs=3) as fp,
        ):
            # ---- input DMAs, spread across issue engines so DGE setups
            # overlap and the projection starts early ----
            wb_sb = cp.tile([P, P], b16)
            nc.scalar.dma_start(wb_sb[:, :], wb[:, :])
            # xtb in two large pieces, one per HWDGE ring, so the transfers
            # overlap and projection is fed without per-chunk stalls
            xtb_chunks = []
            for g in range(ngrp):
                w = min(512, nblk * P - g * 512)
                ch = cp.tile([P, w], b16, name=f"xtb{g}", tag=f"xtb{g}")
                nc.sync.dma_start(ch[:, :], xtb[:, g * 512 : g * 512 + w])
                xtb_chunks.append(ch)

            def xtb_blk(blk):
                return xtb_chunks[blk // 4][:, (blk % 4) * P : (blk % 4) * P + P]

            rb_sb = cp.tile([P, HALF], b16)
            nc.scalar.dma_start(rb_sb[:, :], rb[:, :])
            ef_sb = cp.tile([P, 2 * nblk], f32)
            nc.gpsimd.dma_start(ef_sb[:, :], ef[:, :])
            cst_sb = cp.tile([8, 4 * P], b16)
            nc.gpsimd.dma_start(cst_sb[:, :], cst[:, :])
            cmv_sb = cp.tile([8, HALF], b16)
            nc.gpsimd.dma_start(cmv_sb[:, :], cmv[:, :])
            ub_sb = cp.tile([P, HALF], b16)
            nc.gpsimd.dma_start(ub_sb[:, :], ub[:, :])
            xtd_sb = cp.tile([P, HALF], f32)
            nc.gpsimd.dma_start(xtd_sb[:, :], xtd[:, :])



            wxs = cp.tile([P, nblk * P], b16)

            with (
                tc.tile_pool(name="pproj", bufs=2, space="PSUM") as pp,
                tc.tile_pool(name="pnum", bufs=1, space="PSUM") as pn,
            ):
                # ---- num^T accumulator: 4 psum banks ----
                num = [pn.tile([P, 512], f32, name=f"num{q}") for q in range(4)]

                # ---- projection: wx rows per block (4 blocks / bank);
                # evacuate on ACT so the DVE is free for et tiles ----
                for g in range(ngrp):
                    ps = pp.tile([P, 512], f32, name="projps", tag="pj")
                    for c in range(min(4, nblk - 4 * g)):
                        nc.tensor.matmul(
                            ps[:, c * P : (c + 1) * P],
                            xtb_blk(4 * g + c),
                            wb_sb[:, :],
                            start=True,
                            stop=True,
                        )
                    w = min(512, (nblk - 4 * g) * P)
                    nc.scalar.activation(
                        wxs[:, g * 512 : g * 512 + w], ps[:, :w], Act.Copy
                    )

                # ---- bank-major: per psum bank, coarse rank-8 matmul
                # (start=True initializes the bank), band segments, then the
                # finalize chain -- so bank q's output streams out while
                # bank q+1 still computes ----
                et_tiles = {}
                for blk in sorted(users):
                    jlo, jhi = users[blk]
                    et_tiles[blk] = (jlo * P, (jhi + 1) * P)

                for q in range(4):
                    nc.tensor.matmul(
                        num[q][:, :],
                        cst_sb[:, q * P : (q + 1) * P],
                        cmv_sb[:, q * 512 : (q + 1) * 512],
                        start=True,
                        stop=False,
                        skip_group_check=True,
                    )
                    # blocks whose tile span intersects this bank
                    q_lo, q_hi = q * 512, (q + 1) * 512
                    blks = [
                        b for b, (lo_n, hi_n) in et_tiles.items()
                        if lo_n < q_hi and hi_n > q_lo
                    ]
                    for bi, blk in enumerate(blks):
                        lo_n, hi_n = et_tiles[blk]
                        c0, c1 = max(lo_n, q_lo), min(hi_n, q_hi)
                        et = ep.tile([P, 512], b16, name="et")
                        nc.vector.tensor_scalar(
                            et[:, : c1 - c0],
                            rb_sb[:, c0:c1],
                            ef_sb[:, nblk + blk : nblk + blk + 1],
                            ef_sb[:, blk : blk + 1],
                            Alu.mult,
                            Alu.max,
                        )
                        nc.tensor.matmul(
                            num[q][:, c0 - q_lo : c1 - q_lo],
                            wxs[:, blk * P : (blk + 1) * P],
                            et[:, : c1 - c0],
                            start=False,
                            stop=(bi == len(blks) - 1),
                            skip_group_check=True,
                        )

                    # finalize this bank: out^T = xtd - lrelu(num)*u
                    # (u > 0 so lrelu and the u-scale commute; lrelu on DVE
                    # as (num*0.01) max num -- no ACT table switch)
                    sl = slice(q * 512, (q + 1) * 512)
                    agg = fp.tile([P, 512], f32, name="agg", tag="agg")
                    nc.vector.tensor_tensor(
                        agg[:, :], num[q][:, :], ub_sb[:, sl], Alu.mult
                    )
                    m1 = fp.tile([P, 512], f32, name="m1", tag="m1")
                    nc.scalar.activation(m1[:, :], agg[:, :], Act.Lrelu, alpha=0.01)
                    o_t = fp.tile([P, 512], f32, name="o", tag="o")
                    sub_eng = nc.gpsimd if q % 2 == 0 else nc.vector
                    sub_eng.tensor_tensor(
                        o_t[:, :], xtd_sb[:, sl], m1[:, :], Alu.subtract
                    )
                    eng = nc.sync if q % 2 == 0 else nc.scalar
                    eng.dma_start(out[:, sl], o_t[:, :])

    nc.compile()
    return nc


def _prep(x, W, a1, a2):
    """Host prep: sorting, prefix tables, packing. Returns (in_maps,
    sched, scatter) where scatter[c] = dest node ids per core."""
    x = np.asarray(x, dtype=np.float32)
    W = np.asarray(W, dtype=np.float32)
    a1 = np.asarray(a1, dtype=np.float32)
    a2 = np.asarray(a2, dtype=np.float32)
    w1 = W @ a1 / SCALE
    w2 = W @ a2 / SCALE
    Wb16 = W.astype(_bf16)

    cores = []
    kmins = np.zeros((8, NTILE), dtype=np.int64)
    kmaxs = np.zeros((8, NTILE), dtype=np.int64)
    for t in range(T):
        xt = x[:, t, :]
        s1 = xt @ w1
        s2 = xt @ w2
        tau = -s2
        for h in range(2):
            c = 2 * t + h
            if h == 0:
                order = np.argsort(s1, kind="stable")
            else:
                order = np.argsort(-s1, kind="stable")
            s1s = s1[order]
            if h == 0:
                kfull = np.searchsorted(s1s, tau, side="left")
            else:
                kfull = np.searchsorted(-s1s, -tau, side="left")
            trank = np.argsort(tau, kind="stable")
            dest = trank[:HALF] if h == 0 else trank[HALF:]
            k = kfull[dest]
            dsort = dest[np.argsort(k, kind="stable")]
            k = kfull[dsort]
            for j in range(NTILE):
                kk = k[j * P : (j + 1) * P]
                kmins[c, j] = kk.min()
                kmaxs[c, j] = kk.max()
            cores.append((c, h, order, s1s, dsort, k, xt, s2))

    sched = _schedule(kmins, kmaxs)
    nblk = max(b + cc for b, cc in sched)

    in_maps = [None] * 8
    scatter = [None] * 8
    for (c, h, order, s1s, dsort, k, xt, s2) in cores:
        E1 = np.exp(s1s)
        F1 = np.exp(0.01 * s1s)
        xs = xt[order]                                  # [N, D] sorted
        # coarse prefix tables in f64 for exactness
        GX = np.cumsum(E1[:, None] * xs, axis=0, dtype=np.float64)
        HX = np.cumsum(F1[:, None] * xs, axis=0, dtype=np.float64)
        GX = np.vstack([np.zeros((1, D)), GX])
        HX = np.vstack([np.zeros((1, D)), HX])
        ge = np.concatenate([[0.0], np.cumsum(E1, dtype=np.float64)])
        fe = np.concatenate([[0.0], np.cumsum(F1, dtype=np.float64)])

        r = np.exp(-0.99 * s2[dsort])                   # [2048]
        # exact den on host: for h=0, m<k -> F1*r ; else E1
        if h == 0:
            den = fe[k] * r + (ge[N] - ge[k])
        else:
            den = ge[k] + (fe[N] - fe[k]) * r
        u = 1.0 / den

        # coarse rows per tile at the static band edges, packed for one
        # rank-8 matmul per psum bank: cst[2*jj:2*jj+2, bank*P:...] holds
        # tile (4*bank+jj)'s (g_row, h_row); cmv rows 2*jj:2*jj+2 are
        # (ones, r) on that tile's columns and zero elsewhere.
        cst_arr = np.zeros((8, 4 * P), dtype=np.float64)
        cmv_arr = np.zeros((8, HALF), dtype=np.float32)
        for j, (b, cc) in enumerate(sched):
            lo, hi = P * b, P * (b + cc)
            kk = k[j * P : (j + 1) * P]
            assert kk.min() >= lo and kk.max() <= hi, (c, j, lo, hi, kk.min(), kk.max())
            if h == 0:
                g_row = (GX[N] - GX[hi]) @ W            # E1 branch above band
                h_row = HX[lo] @ W                      # F1 branch below band
            else:
                g_row = GX[lo] @ W
                h_row = (HX[N] - HX[hi]) @ W
            q, jj = j // 4, j % 4
            cst_arr[2 * jj, q * P : (q + 1) * P] = g_row
            cst_arr[2 * jj + 1, q * P : (q + 1) * P] = h_row
            cmv_arr[2 * jj, j * P : (j + 1) * P] = 1.0
            cmv_arr[2 * jj + 1, j * P : (j + 1) * P] = r[j * P : (j + 1) * P]

        xtb = np.zeros((P, nblk * P), dtype=_bf16)
        xtb[:, : nblk * P] = xs[: nblk * P].T.astype(_bf16)
        ef = np.zeros((P, 2 * nblk), dtype=np.float32)
        for blk in range(nblk):
            ef[:, blk] = E1[blk * P : (blk + 1) * P]
            ef[:, nblk + blk] = F1[blk * P : (blk + 1) * P]
        rb_arr = np.ascontiguousarray(np.broadcast_to(r.astype(_bf16), (P, HALF)))
        ub_arr = np.ascontiguousarray(np.broadcast_to(u.astype(_bf16), (P, HALF)))
        xtd = np.ascontiguousarray(xt[dsort].T)

        in_maps[c] = {
            "xtb": xtb,
            "wb": np.ascontiguousarray(Wb16),
            "ef": ef,
            "rb": rb_arr,
            "ub": ub_arr,
            "cst": cst_arr.astype(_bf16),
            "cmv": cmv_arr.astype(_bf16),
            "xtd": xtd,
        }
        scatter[c] = dsort
    return in_maps, sched, scatter


def _run(x, W, a1, a2, trace=False):
    from concourse.bass_utils import run_bass_kernel_spmd

    in_maps, sched, scatter = _prep(x, W, a1, a2)
    if sched not in _CACHE:
        _CACHE[sched] = _build(sched)
    nc = _CACHE[sched]
    res = run_bass_kernel_spmd(nc, in_maps, list(range(8)), trace=trace)
    out_full = np.empty((N, T, D), dtype=np.float32)
    for c in range(8):
        t = c // 2
        out_full[scatter[c], t, :] = np.asarray(
            res.results[c]["out"], dtype=np.float32
        ).T
    return out_full, res


def kernel(x, W, a1, a2):
    out, _ = _run(x, W, a1, a2, trace=False)
    return out


# revision 4
# speedup vs baseline: 1.0863x; 1.0586x over previous
"""Band-decomposition GAT kernel for 8 trn2 NeuronCores.

Math (reference):
    Wx = x @ W;  s1 = Wx@a1/s;  s2 = Wx@a2/s   (s = sqrt(2D), per t)
    weight = softmax_m(lrelu(s1[m] + s2[n]));  agg = lrelu(weight @ Wx)
    out = x - agg

Key identities (per t):
  * Rescaling the softmax row by exp(-s2[n]):
        E~[m,n] = max(E1[m], F1[m] * r[n]),
    E1 = exp(s1), F1 = exp(0.01 s1), r = exp(-0.99 s2); the branch flips
    exactly at s1[m] >= -s2[n] =: tau[n].
  * With m SORTED by s1 and dest n sorted by tau, k(n) = searchsorted
    (s1_sorted, tau[n]) is monotone: all m below k are in the F1*r branch,
    all above in the E1 branch. For a 128-dest tile whose k-range fits in a
    static window of blocks [B_j, B_j+C_j):
        num[n,:] = r[n]*Hpre[128 B_j] + Gsuf[128(B_j+C_j)]
                   + sum_{in band} max(E1, F1 r[n]) Wx[m,:]
    where Gpre/Hpre/Gsuf are prefix/suffix sums of E1*Wx / F1*Wx over
    sorted m -- and since prefix commutes with @W, the HOST computes the
    coarse rows exactly (they are (cumsum of E1*x) @ W).
  * den[n] is a pure function of (s1, s2): host computes u = 1/den.
  * Odd cores take the HIGH-tau dest half with the m-axis sort MIRRORED,
    so the same static band schedule serves all 8 SPMD cores.

Device per core (t, half): project the sorted band blocks (Wx), build the
dense band scores with one fused DVE op per block span, accumulate
num^T[d, n] via per-block matmuls + one rank-2 matmul per tile for the
coarse terms, then out^T = xdest^T - lrelu(num^T * u).  Host transposes
and unpermutes the [128, 2048] per-core result.
"""

import sys

if "/opt/trn_rl_repo" not in sys.path:
    sys.path.insert(0, "/opt/trn_rl_repo")

import numpy as np
import ml_dtypes

_bf16 = ml_dtypes.bfloat16

N, T, D = 4096, 4, 128
P = 128
HALF = N // 2
NTILE = HALF // P          # 16 dest tiles per core
SCALE = (2.0 * D) ** 0.5

_CACHE = {}


def _schedule(kmins, kmaxs):
    """Static per-tile band windows from cross-core k ranges.

    kmins/kmaxs: [8, NTILE] arrays. Returns tuple of (B_j, C_j)."""
    lo = kmins.min(axis=0)
    hi = kmaxs.max(axis=0)
    sched = []
    for j in range(NTILE):
        b = int(lo[j]) // P
        c = -(-int(hi[j]) // P) - b  # ceil
        c = max(c, 1)
        sched.append((b, c))
    return tuple(sched)


def _build(sched):
    import concourse.mybir as mybir
    from concourse import bacc
    from concourse.tile import TileContext

    f32 = mybir.dt.float32
    b16 = mybir.dt.bfloat16
    Alu = mybir.AluOpType
    Act = mybir.ActivationFunctionType

    nblk = max(b + c for b, c in sched)
    # block B -> contiguous run of tiles [jlo, jhi] that use it
    users = {}
    for j, (b, c) in enumerate(sched):
        for bb in range(b, b + c):
            lo, hi = users.get(bb, (j, j))
            users[bb] = (min(lo, j), max(hi, j))

    nc = bacc.Bacc()
    xtb = nc.declare_dram_parameter("xtb", [P, nblk * P], b16, isOutput=False)
    wb = nc.declare_dram_parameter("wb", [P, P], b16, isOutput=False)
    ef = nc.declare_dram_parameter("ef", [P, 2 * nblk], f32, isOutput=False)
    rb = nc.declare_dram_parameter("rb", [P, HALF], b16, isOutput=False)
    ub = nc.declare_dram_parameter("ub", [P, HALF], b16, isOutput=False)
    cst = nc.declare_dram_parameter("cst", [8, 4 * P], b16, isOutput=False)
    cmv = nc.declare_dram_parameter("cmv", [8, HALF], b16, isOutput=False)
    xtd = nc.declare_dram_parameter("xtd", [P, HALF], f32, isOutput=False)
    out = nc.declare_dram_parameter("out", [P, HALF], f32, isOutput=True)

    ngrp = -(-nblk // 4)

    with TileContext(nc) as tc:
        with (
            tc.tile_pool(name="const", bufs=1) as cp,
            tc.tile_pool(name="et", bufs=8) as ep,
            tc.tile_pool(name="fin", bufs=3) as fp,
        ):
            # ---- input DMAs, spread across issue engines so DGE setups
            # overlap and the projection starts early ----
            wb_sb = cp.tile([P, P], b16)
            nc.scalar.dma_start(wb_sb[:, :], wb[:, :])
            # xtb in two large pieces, one per HWDGE ring, so the transfers
            # overlap and projection is fed without per-chunk stalls
            xtb_chunks = []
            for g in range(ngrp):
                w = min(512, nblk * P - g * 512)
                ch = cp.tile([P, w], b16, name=f"xtb{g}", tag=f"xtb{g}")
                nc.sync.dma_start(ch[:, :], xtb[:, g * 512 : g * 512 + w])
                xtb_chunks.append(ch)

            def xtb_blk(blk):
                return xtb_chunks[blk // 4][:, (blk % 4) * P : (blk % 4) * P + P]

            rb_sb = cp.tile([P, HALF], b16)
            nc.scalar.dma_start(rb_sb[:, :], rb[:, :])
            ef_sb = cp.tile([P, 2 * nblk], f32)
            nc.gpsimd.dma_start(ef_sb[:, :], ef[:, :])
            cst_sb = cp.tile([8, 4 * P], b16)
            nc.gpsimd.dma_start(cst_sb[:, :], cst[:, :])
            cmv_sb = cp.tile([8, HALF], b16)
            nc.gpsimd.dma_start(cmv_sb[:, :], cmv[:, :])
            ub_sb = cp.tile([P, HALF], b16)
            nc.gpsimd.dma_start(ub_sb[:, :], ub[:, :])
            xtd_sb = cp.tile([P, HALF], f32)
            nc.gpsimd.dma_start(xtd_sb[:, :], xtd[:, :])



            wxs = cp.tile([P, nblk * P], b16)

            with (
                tc.tile_pool(name="pproj", bufs=2, space="PSUM") as pp,
                tc.tile_pool(name="pnum", bufs=1, space="PSUM") as pn,
            ):
                # ---- num^T accumulator: 4 psum banks ----
                num = [pn.tile([P, 512], f32, name=f"num{q}") for q in range(4)]

                # ---- projection groups (4 blocks / bank), emitted lazily
                # so ACT's program order interleaves evacuations with the
                # per-bank lrelus; evacuate on ACT, DVE stays free for et ----
                emitted = [False] * ngrp

                def emit_group(g):
                    if emitted[g]:
                        return
                    emitted[g] = True
                    ps = pp.tile([P, 512], f32, name="projps", tag="pj")
                    for c in range(min(4, nblk - 4 * g)):
                        nc.tensor.matmul(
                            ps[:, c * P : (c + 1) * P],
                            xtb_blk(4 * g + c),
                            wb_sb[:, :],
                            start=True,
                            stop=True,
                        )
                    w = min(512, (nblk - 4 * g) * P)
                    nc.scalar.activation(
                        wxs[:, g * 512 : g * 512 + w], ps[:, :w], Act.Copy
                    )

                # ---- bank-major: per psum bank, coarse rank-8 matmul
                # (start=True initializes the bank), band segments, then the
                # finalize chain -- so bank q's output streams out while
                # bank q+1 still computes ----
                et_tiles = {}
                for blk in sorted(users):
                    jlo, jhi = users[blk]
                    et_tiles[blk] = (jlo * P, (jhi + 1) * P)

                for q in range(4):
                    q_max_blk = max(
                        b for b, (lo_n, hi_n) in et_tiles.items()
                        if lo_n < (q + 1) * 512 and hi_n > q * 512
                    )
                    for g in range(q_max_blk // 4 + 1):
                        emit_group(g)
                    nc.tensor.matmul(
                        num[q][:, :],
                        cst_sb[:, q * P : (q + 1) * P],
                        cmv_sb[:, q * 512 : (q + 1) * 512],
                        start=True,
                        stop=False,
                        skip_group_check=True,
                    )
                    # blocks whose tile span intersects this bank
                    q_lo, q_hi = q * 512, (q + 1) * 512
                    blks = [
                        b for b, (lo_n, hi_n) in et_tiles.items()
                        if lo_n < q_hi and hi_n > q_lo
                    ]
                    for bi, blk in enumerate(blks):
                        lo_n, hi_n = et_tiles[blk]
                        c0, c1 = max(lo_n, q_lo), min(hi_n, q_hi)
                        et = ep.tile([P, 512], b16, name="et")
                        nc.vector.tensor_scalar(
                            et[:, : c1 - c0],
                            rb_sb[:, c0:c1],
                            ef_sb[:, nblk + blk : nblk + blk + 1],
                            ef_sb[:, blk : blk + 1],
                            Alu.mult,
                            Alu.max,
                        )
                        nc.tensor.matmul(
                            num[q][:, c0 - q_lo : c1 - q_lo],
                            wxs[:, blk * P : (blk + 1) * P],
                            et[:, : c1 - c0],
                            start=False,
                            stop=(bi == len(blks) - 1),
                            skip_group_check=True,
                        )

                    # finalize this bank: out^T = xtd - lrelu(num)*u
                    # (u > 0 so lrelu and the u-scale commute; lrelu on DVE
                    # as (num*0.01) max num -- no ACT table switch)
                    sl = slice(q * 512, (q + 1) * 512)
                    agg = fp.tile([P, 512], f32, name="agg", tag="agg")
                    nc.vector.tensor_tensor(
                        agg[:, :], num[q][:, :], ub_sb[:, sl], Alu.mult
                    )
                    m1 = fp.tile([P, 512], f32, name="m1", tag="m1")
                    nc.scalar.activation(m1[:, :], agg[:, :], Act.Lrelu, alpha=0.01)
                    o_t = fp.tile([P, 512], f32, name="o", tag="o")
                    sub_eng = nc.gpsimd if q % 2 == 0 else nc.vector
                    sub_eng.tensor_tensor(
                        o_t[:, :], xtd_sb[:, sl], m1[:, :], Alu.subtract
                    )
                    eng = nc.sync if q % 2 == 0 else nc.scalar
                    eng.dma_start(out[:, sl], o_t[:, :])

    nc.compile()
    return nc


def _prep(x, W, a1, a2):
    """Host prep: sorting, prefix tables, packing. Returns (in_maps,
    sched, scatter) where scatter[c] = dest node ids per core."""
    x = np.asarray(x, dtype=np.float32)
    W = np.asarray(W, dtype=np.float32)
    a1 = np.asarray(a1, dtype=np.float32)
    a2 = np.asarray(a2, dtype=np.float32)
    w1 = W @ a1 / SCALE
    w2 = W @ a2 / SCALE
    Wb16 = W.astype(_bf16)

    cores = []
    kmins = np.zeros((8, NTILE), dtype=np.int64)
    kmaxs = np.zeros((8, NTILE), dtype=np.int64)
    for t in range(T):
        xt = x[:, t, :]
        s1 = xt @ w1
        s2 = xt @ w2
        tau = -s2
        for h in range(2):
            c = 2 * t + h
            if h == 0:
                order = np.argsort(s1, kind="stable")
            else:
                order = np.argsort(-s1, kind="stable")
            s1s = s1[order]
            if h == 0:
                kfull = np.searchsorted(s1s, tau, side="left")
            else:
                kfull = np.searchsorted(-s1s, -tau, side="left")
            trank = np.argsort(tau, kind="stable")
            dest = trank[:HALF] if h == 0 else trank[HALF:]
            k = kfull[dest]
            dsort = dest[np.argsort(k, kind="stable")]
            k = kfull[dsort]
            for j in range(NTILE):
                kk = k[j * P : (j + 1) * P]
                kmins[c, j] = kk.min()
                kmaxs[c, j] = kk.max()
            cores.append((c, h, order, s1s, dsort, k, xt, s2))

    sched = _schedule(kmins, kmaxs)
    nblk = max(b + cc for b, cc in sched)

    in_maps = [None] * 8
    scatter = [None] * 8
    for (c, h, order, s1s, dsort, k, xt, s2) in cores:
        E1 = np.exp(s1s)
        F1 = np.exp(0.01 * s1s)
        xs = xt[order]                                  # [N, D] sorted
        # coarse prefix tables in f64 for exactness
        GX = np.cumsum(E1[:, None] * xs, axis=0, dtype=np.float64)
        HX = np.cumsum(F1[:, None] * xs, axis=0, dtype=np.float64)
        GX = np.vstack([np.zeros((1, D)), GX])
        HX = np.vstack([np.zeros((1, D)), HX])
        ge = np.concatenate([[0.0], np.cumsum(E1, dtype=np.float64)])
        fe = np.concatenate([[0.0], np.cumsum(F1, dtype=np.float64)])

        r = np.exp(-0.99 * s2[dsort])                   # [2048]
        # exact den on host: for h=0, m<k -> F1*r ; else E1
        if h == 0:
            den = fe[k] * r + (ge[N] - ge[k])
        else:
            den = ge[k] + (fe[N] - fe[k]) * r
        u = 1.0 / den

        # coarse rows per tile at the static band edges, packed for one
        # rank-8 matmul per psum bank: cst[2*jj:2*jj+2, bank*P:...] holds
        # tile (4*bank+jj)'s (g_row, h_row); cmv rows 2*jj:2*jj+2 are
        # (ones, r) on that tile's columns and zero elsewhere.
        cst_arr = np.zeros((8, 4 * P), dtype=np.float64)
        cmv_arr = np.zeros((8, HALF), dtype=np.float32)
        for j, (b, cc) in enumerate(sched):
            lo, hi = P * b, P * (b + cc)
            kk = k[j * P : (j + 1) * P]
            assert kk.min() >= lo and kk.max() <= hi, (c, j, lo, hi, kk.min(), kk.max())
            if h == 0:
                g_row = (GX[N] - GX[hi]) @ W            # E1 branch above band
                h_row = HX[lo] @ W                      # F1 branch below band
            else:
                g_row = GX[lo] @ W
                h_row = (HX[N] - HX[hi]) @ W
            q, jj = j // 4, j % 4
            cst_arr[2 * jj, q * P : (q + 1) * P] = g_row
            cst_arr[2 * jj + 1, q * P : (q + 1) * P] = h_row
            cmv_arr[2 * jj, j * P : (j + 1) * P] = 1.0
            cmv_arr[2 * jj + 1, j * P : (j + 1) * P] = r[j * P : (j + 1) * P]

        xtb = np.zeros((P, nblk * P), dtype=_bf16)
        xtb[:, : nblk * P] = xs[: nblk * P].T.astype(_bf16)
        ef = np.zeros((P, 2 * nblk), dtype=np.float32)
        for blk in range(nblk):
            ef[:, blk] = E1[blk * P : (blk + 1) * P]
            ef[:, nblk + blk] = F1[blk * P : (blk + 1) * P]
        rb_arr = np.ascontiguousarray(np.broadcast_to(r.astype(_bf16), (P, HALF)))
        ub_arr = np.ascontiguousarray(np.broadcast_to(u.astype(_bf16), (P, HALF)))
        xtd = np.ascontiguousarray(xt[dsort].T)

        in_maps[c] = {
            "xtb": xtb,
            "wb": np.ascontiguousarray(Wb16),
            "ef": ef,
            "rb": rb_arr,
            "ub": ub_arr,
            "cst": cst_arr.astype(_bf16),
            "cmv": cmv_arr.astype(_bf16),
            "xtd": xtd,
        }
        scatter[c] = dsort
    return in_maps, sched, scatter


def _run(x, W, a1, a2, trace=False):
    from concourse.bass_utils import run_bass_kernel_spmd

    in_maps, sched, scatter = _prep(x, W, a1, a2)
    if sched not in _CACHE:
        _CACHE[sched] = _build(sched)
    nc = _CACHE[sched]
    res = run_bass_kernel_spmd(nc, in_maps, list(range(8)), trace=trace)
    out_full = np.empty((N, T, D), dtype=np.float32)
    for c in range(8):
        t = c // 2
        out_full[scatter[c], t, :] = np.asarray(
            res.results[c]["out"], dtype=np.float32
        ).T
    return out_full, res


def kernel(x, W, a1, a2):
    out, _ = _run(x, W, a1, a2, trace=False)
    return out
